# revision 13
# baseline (speedup 1.0000x reference)
"""Trainium2 Bass kernel for nn_MergingModel (crystallography merging ELBO).

Strategy: data-parallel over the image axis N (8 images -> 8 NeuronCores).
Each core processes one image's 2 reindexing ops x 16384 reflections x 2
harmonics = 65536 observations:
  - per-obs indirect-DMA gather of 144B rows from a combined table
    [N_REFL, 36] = [eps.T (32 MC cols) | loc | raw_scale | pad] (host-packed
    layout; softplus of the gathered raw_scale runs on device)
  - KL over the full posterior from the [N_REFL, 2] (loc, raw) table
  - 13->128->32 MLP per observation on the tensor engine (loc/qs feature
    columns enter via a PE transpose + selector-weight matmul)
  - masked Ipred accumulation over harmonics, per-MC log-likelihood,
    weighted-Pearson partial sums
Host combines the 16 partial sums per core into (elbo, kl, cc, op_idx).
"""
import os
import sys

import numpy as np

try:
    import concourse  # noqa: F401
except ImportError:
    sys.path.insert(0, "/opt/trn_rl_repo")

import concourse.bacc as bacc
import concourse.bass as bass
import concourse.tile as tile
from concourse import mybir
from concourse.bass_utils import run_bass_kernel_spmd

F32 = mybir.dt.float32
I32 = mybir.dt.int32

N_REFL = 262144
N = 8
R = 16384
H = 2
OPS = 2
MC = 32
NSEG = OPS * H  # segment order: (op, h)
LOG2PI = 1.8378770664093453
NCH = R // 128          # 128 obs-chunks per segment
DH = 128                # hidden
EW = 36                 # combined table row width (32 eps + loc + raw + pad)


def _emit(nc: bass.Bass, tc: tile.TileContext, t):
    """Emit the per-core program. `t` is a dict of DRAM tensor handles."""
    from contextlib import ExitStack

    with ExitStack() as ctx:
        ec = ctx.enter_context
        const = ec(tc.tile_pool(name="const", bufs=1))
        lqp = ec(tc.tile_pool(name="lqp", bufs=1))
        fap = ec(tc.tile_pool(name="fap", bufs=3))
        gath = ec(tc.tile_pool(name="gath", bufs=2))
        small = ec(tc.tile_pool(name="small", bufs=2))
        mid = ec(tc.tile_pool(name="mid", bufs=3))
        full = ec(tc.tile_pool(name="full", bufs=1))
        ipool = ec(tc.tile_pool(name="ipool", bufs=1))
        ps1p = ec(tc.tile_pool(name="ps1p", bufs=2, space="PSUM"))
        ps2p = ec(tc.tile_pool(name="ps2p", bufs=2, space="PSUM"))
        ltpsp = ec(tc.tile_pool(name="ltpsp", bufs=2, space="PSUM"))
        accpsp = ec(tc.tile_pool(name="accpsp", bufs=1, space="PSUM"))

        # ---- constants ----
        w1a_t = const.tile([11, DH], F32)
        nc.sync.dma_start(w1a_t[:], t["w1a"].ap())
        w1sel_t = const.tile([32, 16 * 128], F32)
        nc.sync.dma_start(w1sel_t[:], t["w1sel"].ap())
        b1_t = const.tile([DH, 1], F32)
        nc.sync.dma_start(b1_t[:], t["b1"].ap())
        w2_t = const.tile([DH, MC], F32)
        nc.sync.dma_start(w2_t[:], t["w2"].ap())
        b2r_t = const.tile([128, MC], F32)
        nc.sync.dma_start(b2r_t[:], t["b2r"].ap())
        ipm_t = const.tile([128, NCH], F32)
        nc.sync.dma_start(ipm_t[:], t["ipm"].ap())
        spm_t = const.tile([128, NCH], F32)
        nc.sync.dma_start(spm_t[:], t["spm"].ap())
        # identity + ones from host, keeping the Pool engine free for gathers
        id_t = const.tile([128, 128], F32)
        nc.sync.dma_start(id_t[:], t["ident"].ap())
        ones_t = const.tile([128, 1], F32)
        nc.sync.dma_start(ones_t[:], t["ones"].ap())

        acc_t = const.tile([128, 16], F32)
        nc.vector.memset(acc_t[:], 0.0)

        # ---- phase A: KL partials over the full (loc, raw) table ----
        lq_t = lqp.tile([128, 2048, 2], F32)
        nc.sync.dma_start(lq_t[:], t["lq"].ap().rearrange("(p k) t -> p k t", p=128))
        qs_ap = lq_t[:, :, 1]
        # softplus(x) = ln(exp(x) + 1); raw_scale ~ N(0, 0.1) so exp is safe.
        nc.scalar.activation(qs_ap, qs_ap, mybir.ActivationFunctionType.Exp)
        nc.scalar.activation(qs_ap, qs_ap, mybir.ActivationFunctionType.Ln,
                             bias=1.0)
        # kl partials: 0.5*sum(loc^2+qs^2) - 1024 - sum log qs   (per partition)
        sq_t = lqp.tile([128, 2048, 2], F32, tag="sq")
        nc.vector.tensor_tensor(out=sq_t[:], in0=lq_t[:], in1=lq_t[:],
                                op=mybir.AluOpType.mult)
        kl_a = small.tile([128, 1], F32, tag="kl_a")
        nc.vector.tensor_reduce(out=kl_a[:], in_=sq_t[:],
                                axis=mybir.AxisListType.XY, op=mybir.AluOpType.add)
        lnq_t = lqp.tile([128, 2048], F32, tag="lnq")
        kl_b = small.tile([128, 1], F32, tag="kl_b")
        nc.scalar.activation(lnq_t[:], qs_ap, mybir.ActivationFunctionType.Ln,
                             accum_out=kl_b[:])
        kl_tmp = small.tile([128, 1], F32, tag="kl_tmp")
        nc.vector.tensor_scalar(out=kl_tmp[:], in0=kl_a[:], scalar1=0.5,
                                scalar2=-1024.0, op0=mybir.AluOpType.mult,
                                op1=mybir.AluOpType.add)
        nc.vector.tensor_tensor(out=acc_t[:, 4:5], in0=kl_tmp[:], in1=kl_b[:],
                                op=mybir.AluOpType.subtract)

        # ---- derived obs constants ----
        inv_t = full.tile([128, NCH], F32, tag="inv")
        nc.vector.reciprocal(inv_t[:], spm_t[:])
        inv2_t = full.tile([128, NCH], F32, tag="inv2")
        nc.vector.tensor_tensor(out=inv2_t[:], in0=inv_t[:], in1=inv_t[:],
                                op=mybir.AluOpType.mult)
        A_t = full.tile([128, NCH], F32, tag="A")
        nc.vector.tensor_scalar(out=A_t[:], in0=inv2_t[:],
                                scalar1=-1.0 / (2 * MC), scalar2=None,
                                op0=mybir.AluOpType.mult)
        B_t = full.tile([128, NCH], F32, tag="B")
        nc.scalar.activation(B_t[:], spm_t[:], mybir.ActivationFunctionType.Ln)
        nc.vector.tensor_scalar(out=B_t[:], in0=B_t[:], scalar1=-1.0,
                                scalar2=-0.5 * LOG2PI, op0=mybir.AluOpType.mult,
                                op1=mybir.AluOpType.add)

        ll_full = [full.tile([128, NCH], F32, tag=f"ll{o}", name=f"ll{o}")
                   for o in range(OPS)]
        pm_full = [full.tile([128, NCH], F32, tag=f"pm{o}", name=f"pm{o}")
                   for o in range(OPS)]
        obsm = [full.tile([128, NCH], F32, tag=f"obsm{o}", name=f"obsm{o}")
                for o in range(OPS)]

        for op in range(OPS):
            ip_acc = ipool.tile([128, NCH, MC], F32, tag="ip_acc")
            masks = []
            for h in range(H):
                s = op * H + h
                rid_t = small.tile([128, NCH], I32, tag="rid")
                nc.sync.dma_start(rid_t[:], t["ridpm"].ap()[s])
                mask_t = small.tile([128, NCH], F32, tag=f"mask{h}")
                nc.vector.tensor_scalar(out=mask_t[:], in0=rid_t[:], scalar1=0,
                                        scalar2=None, op0=mybir.AluOpType.is_ge)
                masks.append(mask_t)
                ridc_t = small.tile([128, NCH], I32, tag="ridc")
                nc.vector.tensor_scalar(out=ridc_t[:], in0=rid_t[:], scalar1=0,
                                        scalar2=None, op0=mybir.AluOpType.max)
                # per-chunk gathers: one index per partition per instruction
                g36 = gath.tile([128, NCH, EW], F32, tag="g36")
                for c in range(NCH):
                    nc.gpsimd.indirect_dma_start(
                        out=g36[:, c, :], out_offset=None, in_=t["eps36"].ap(),
                        in_offset=bass.IndirectOffsetOnAxis(
                            ap=ridc_t[:, c:c + 1], axis=0))
                # softplus the gathered raw_scale column (-> q_scale)
                qs_col = g36[:, :, 33]
                nc.scalar.activation(qs_col, qs_col,
                                     mybir.ActivationFunctionType.Exp)
                nc.scalar.activation(qs_col, qs_col,
                                     mybir.ActivationFunctionType.Ln, bias=1.0)
                lqm_t = small.tile([128, NCH, 2], F32, tag="lqm")
                nc.vector.tensor_tensor(
                    out=lqm_t[:], in0=g36[:, :, 32:34],
                    in1=mask_t[:].unsqueeze(2).to_broadcast([128, NCH, 2]),
                    op=mybir.AluOpType.mult)

                for g in range(8):  # 2048-obs groups
                    # transpose the MASKED (loc,qs): identical final results —
                    # masked observations' scale is only ever used times z=0
                    lt_ps = ltpsp.tile([32, 128], F32, tag="lt_ps")
                    nc.tensor.transpose(
                        out=lt_ps[:],
                        in_=lqm_t[:, g * 16:(g + 1) * 16, :].rearrange(
                            "p a b -> p (a b)"),
                        identity=id_t[:])
                    lt_sb = mid.tile([32, 128], F32, tag="lt_sb")
                    nc.vector.tensor_copy(lt_sb[:], lt_ps[:])
                    for c4 in range(4):  # 512-obs chunks
                        c = g * 4 + c4
                        q, cq = divmod(c, 8)
                        fa_t = _get_fa(nc, fap, t, s, q)
                        ps1 = ps1p.tile([128, 512], F32, tag="ps1")
                        nc.tensor.matmul(out=ps1[:], lhsT=w1a_t[:],
                                         rhs=fa_t[:, cq * 512:(cq + 1) * 512],
                                         start=True, stop=False,
                                         skip_group_check=True)
                        for j4 in range(4):
                            j = c4 * 4 + j4
                            nc.tensor.matmul(
                                out=ps1[:, j4 * 128:(j4 + 1) * 128],
                                lhsT=w1sel_t[:, j * 128:(j + 1) * 128],
                                rhs=lt_sb[:],
                                start=False, stop=True, skip_group_check=True)
                        hdn_sb = mid.tile([128, 512], F32, tag="hdn")
                        nc.scalar.activation(hdn_sb[:], ps1[:],
                                             mybir.ActivationFunctionType.Relu,
                                             bias=b1_t[:])
                        ps2 = ps2p.tile([128, 4, MC], F32, tag="ps2")
                        for j4 in range(4):
                            nc.tensor.matmul(
                                out=ps2[:, j4, :],
                                lhsT=hdn_sb[:, j4 * 128:(j4 + 1) * 128],
                                rhs=w2_t[:], start=True, stop=True)
                        scale_sb = mid.tile([128, 4, MC], F32, tag="scale")
                        nc.vector.tensor_tensor(
                            out=scale_sb[:], in0=ps2[:],
                            in1=b2r_t[:].unsqueeze(1).to_broadcast([128, 4, MC]),
                            op=mybir.AluOpType.add)
                        # Ipred: z = m*loc + m*qs*eps ; ip += z*scale
                        cs = slice(c * 4, (c + 1) * 4)
                        z_t = mid.tile([128, 4, MC], F32, tag="z")
                        nc.vector.tensor_tensor(
                            out=z_t[:], in0=g36[:, cs, 0:32],
                            in1=lqm_t[:, cs, 1:2].to_broadcast([128, 4, MC]),
                            op=mybir.AluOpType.mult)
                        nc.vector.tensor_tensor(
                            out=z_t[:], in0=z_t[:],
                            in1=lqm_t[:, cs, 0:1].to_broadcast([128, 4, MC]),
                            op=mybir.AluOpType.add)
                        if h == 0:
                            nc.vector.tensor_tensor(
                                out=ip_acc[:, cs, :], in0=z_t[:], in1=scale_sb[:],
                                op=mybir.AluOpType.mult)
                        else:
                            prod_t = mid.tile([128, 4, MC], F32, tag="prod")
                            nc.vector.tensor_tensor(
                                out=prod_t[:], in0=z_t[:], in1=scale_sb[:],
                                op=mybir.AluOpType.mult)
                            nc.vector.tensor_tensor(
                                out=ip_acc[:, cs, :], in0=ip_acc[:, cs, :],
                                in1=prod_t[:], op=mybir.AluOpType.add)
            # obs mask for this op
            nc.vector.tensor_tensor(out=obsm[op][:], in0=masks[0][:],
                                    in1=masks[1][:], op=mybir.AluOpType.max)
            # ---- log-likelihood + Ipred mean ----
            for k in range(4):  # 32-chunk slices
                ks = slice(k * 32, (k + 1) * 32)
                d_t = ipool.tile([128, 32, MC], F32, tag="d")
                nc.vector.tensor_tensor(
                    out=d_t[:], in0=ip_acc[:, ks, :],
                    in1=ipm_t[:, ks].unsqueeze(2).to_broadcast([128, 32, MC]),
                    op=mybir.AluOpType.subtract)
                nc.vector.tensor_tensor(out=d_t[:], in0=d_t[:], in1=d_t[:],
                                        op=mybir.AluOpType.mult)
                s2_t = small.tile([128, 32], F32, tag="s2")
                nc.vector.tensor_reduce(out=s2_t[:], in_=d_t[:],
                                        axis=mybir.AxisListType.X,
                                        op=mybir.AluOpType.add)
                nc.vector.tensor_tensor(out=ll_full[op][:, ks], in0=s2_t[:],
                                        in1=A_t[:, ks], op=mybir.AluOpType.mult)
                nc.vector.tensor_tensor(out=ll_full[op][:, ks],
                                        in0=ll_full[op][:, ks], in1=B_t[:, ks],
                                        op=mybir.AluOpType.add)
                nc.vector.tensor_reduce(out=pm_full[op][:, ks],
                                        in_=ip_acc[:, ks, :],
                                        axis=mybir.AxisListType.X,
                                        op=mybir.AluOpType.add)

        # ---- endgame: partial sums into acc columns ----
        scr = full.tile([128, NCH], F32, tag="scr")
        scr2 = full.tile([128, NCH], F32, tag="scr2")

        def red(col, src_ap):
            nc.vector.tensor_reduce(out=acc_t[:, col:col + 1], in_=src_ap,
                                    axis=mybir.AxisListType.X,
                                    op=mybir.AluOpType.add)

        for op in range(OPS):
            nc.vector.tensor_tensor(out=scr[:], in0=ll_full[op][:],
                                    in1=obsm[op][:], op=mybir.AluOpType.mult)
            red(op, scr[:])
            red(2 + op, obsm[op][:])
        w_t = full.tile([128, NCH], F32, tag="w")
        nc.vector.tensor_tensor(out=w_t[:], in0=obsm[1][:], in1=inv2_t[:],
                                op=mybir.AluOpType.mult)
        red(5, w_t[:])
        nc.vector.tensor_tensor(out=scr[:], in0=w_t[:], in1=ipm_t[:],
                                op=mybir.AluOpType.mult)
        red(6, scr[:])
        nc.vector.tensor_tensor(out=scr2[:], in0=scr[:], in1=ipm_t[:],
                                op=mybir.AluOpType.mult)
        red(7, scr2[:])
        for op in range(OPS):
            nc.vector.tensor_scalar(out=pm_full[op][:], in0=pm_full[op][:],
                                    scalar1=1.0 / MC, scalar2=None,
                                    op0=mybir.AluOpType.mult)
            nc.vector.tensor_tensor(out=scr[:], in0=w_t[:], in1=pm_full[op][:],
                                    op=mybir.AluOpType.mult)
            red(8 + 3 * op, scr[:])
            nc.vector.tensor_tensor(out=scr2[:], in0=scr[:], in1=pm_full[op][:],
                                    op=mybir.AluOpType.mult)
            red(9 + 3 * op, scr2[:])
            nc.vector.tensor_tensor(out=scr2[:], in0=scr[:], in1=ipm_t[:],
                                    op=mybir.AluOpType.mult)
            red(10 + 3 * op, scr2[:])

        accps = accpsp.tile([1, 16], F32)
        nc.tensor.matmul(out=accps[:], lhsT=ones_t[:], rhs=acc_t[:],
                         start=True, stop=True)
        accsb = small.tile([1, 16], F32, tag="accsb")
        nc.vector.tensor_copy(accsb[:], accps[:])
        nc.sync.dma_start(t["acc_out"].ap(), accsb[:])


_FA_CACHE = {}


def _get_fa(nc, fap, t, s, q):
    key = (s, q)
    if key not in _FA_CACHE:
        fa_t = fap.tile([11, 4096], F32, tag="fa")
        nc.sync.dma_start(fa_t[:], t["fa"].ap()[s, :, q * 4096:(q + 1) * 4096])
        _FA_CACHE[key] = fa_t
    return _FA_CACHE[key]


_PROGRAM = None


def _build_program():
    global _PROGRAM
    if _PROGRAM is not None:
        return _PROGRAM
    nc = bacc.Bacc("TRN2", target_bir_lowering=False, debug=False)
    t = {}
    t["eps36"] = nc.dram_tensor("eps36", [N_REFL, EW], F32, kind="ExternalInput")
    t["lq"] = nc.dram_tensor("lq", [N_REFL, 2], F32, kind="ExternalInput")
    t["ridpm"] = nc.dram_tensor("ridpm", [NSEG, 128, NCH], I32, kind="ExternalInput")
    t["fa"] = nc.dram_tensor("fa", [NSEG, 11, R], F32, kind="ExternalInput")
    t["ipm"] = nc.dram_tensor("ipm", [128, NCH], F32, kind="ExternalInput")
    t["spm"] = nc.dram_tensor("spm", [128, NCH], F32, kind="ExternalInput")
    t["w1a"] = nc.dram_tensor("w1a", [11, DH], F32, kind="ExternalInput")
    t["w1sel"] = nc.dram_tensor("w1sel", [32, 16 * 128], F32, kind="ExternalInput")
    t["b1"] = nc.dram_tensor("b1", [DH, 1], F32, kind="ExternalInput")
    t["w2"] = nc.dram_tensor("w2", [DH, MC], F32, kind="ExternalInput")
    t["b2r"] = nc.dram_tensor("b2r", [128, MC], F32, kind="ExternalInput")
    t["ident"] = nc.dram_tensor("ident", [128, 128], F32, kind="ExternalInput")
    t["ones"] = nc.dram_tensor("ones", [128, 1], F32, kind="ExternalInput")
    t["acc_out"] = nc.dram_tensor("acc_out", [1, 16], F32, kind="ExternalOutput")

    _FA_CACHE.clear()
    with tile.TileContext(nc) as tc:
        _emit(nc, tc, t)
    nc.compile()
    _PROGRAM = nc
    return nc


def host_prep(inputs):
    eps = np.asarray(inputs["eps"], np.float32)
    loc = np.asarray(inputs["loc"], np.float32)
    raw = np.asarray(inputs["raw_scale"], np.float32)
    eps36 = np.zeros((N_REFL, EW), np.float32)
    eps36[:, 0:MC] = eps.T
    eps36[:, 32] = loc
    eps36[:, 33] = raw
    lq = np.ascontiguousarray(np.stack([loc, raw], -1))
    W1 = np.asarray(inputs["W1"], np.float32)
    w1a = np.ascontiguousarray(W1[[2, 3, 4, 5, 6, 7, 8, 9, 10, 11, 13]])
    w1sel = np.zeros((32, 16 * 128), np.float32)
    for j in range(16):
        w1sel[2 * j, j * 128:(j + 1) * 128] = W1[0]
        w1sel[2 * j + 1, j * 128:(j + 1) * 128] = W1[1]
    b1 = np.ascontiguousarray(np.asarray(inputs["b1"], np.float32).reshape(DH, 1))
    w2 = np.asarray(inputs["W2"], np.float32)
    b2r = np.ascontiguousarray(
        np.broadcast_to(np.asarray(inputs["b2"], np.float32), (128, MC)))

    I_all = np.asarray(inputs["I"], np.float32)
    Sig_all = np.asarray(inputs["SigI"], np.float32)
    meta_all = np.asarray(inputs["metadata"], np.float32)
    rid_all = np.asarray(inputs["refl_id"], np.int32)
    wav_all = np.asarray(inputs["wav_exp"], np.float32)

    maps = []
    for n in range(N):
        I = I_all[n, :, 0]
        SigI = Sig_all[n, :, 0]
        meta = meta_all[n]
        rid = rid_all[:, n, :, :, 0]
        wav = wav_all[:, n, :, :, 0]
        rid_seg = rid.transpose(0, 2, 1).reshape(NSEG, R)
        wav_seg = wav.transpose(0, 2, 1).reshape(NSEG, R)
        rid_pm = np.ascontiguousarray(
            rid_seg.reshape(NSEG, NCH, 128).transpose(0, 2, 1))
        ipm = np.ascontiguousarray(I.reshape(NCH, 128).T)
        spm = np.ascontiguousarray(SigI.reshape(NCH, 128).T)
        fa = np.empty((NSEG, 11, R), np.float32)
        for s in range(NSEG):
            fa[s, 0] = I
            fa[s, 1] = SigI
            fa[s, 2:10] = meta.T
            fa[s, 10] = wav_seg[s]
        maps.append(dict(eps36=eps36, lq=lq, ridpm=rid_pm, fa=fa, ipm=ipm,
                         spm=spm, w1a=w1a, w1sel=w1sel, b1=b1, w2=w2, b2r=b2r,
                         ident=np.eye(128, dtype=np.float32),
                         ones=np.ones((128, 1), np.float32)))
    return maps


def host_combine(accs):
    accs = np.stack([np.asarray(a, np.float64).reshape(16) for a in accs])
    ll_img = accs[:, 0:2] / accs[:, 2:4]
    op_idx = (ll_img[:, 1] > ll_img[:, 0]).astype(np.int32)
    ll_best = np.max(ll_img, axis=1)
    kl = accs[0, 4] / N_REFL
    elbo = -ll_best.mean() + kl
    sel = op_idx == 1
    Sw = accs[:, 5].sum()
    Swx = accs[:, 6].sum()
    Swxx = accs[:, 7].sum()
    Swy = np.where(sel, accs[:, 11], accs[:, 8]).sum()
    Swyy = np.where(sel, accs[:, 12], accs[:, 9]).sum()
    Swxy = np.where(sel, accs[:, 13], accs[:, 10]).sum()
    z = 1.0 / Sw
    mx, my = z * Swx, z * Swy
    cxy = z * Swxy - mx * my
    cx = z * Swxx - mx * mx
    cy = z * Swyy - my * my
    cc = cxy / np.sqrt(cx * cy)
    return (np.float32(elbo), np.float32(kl), np.float32(cc),
            op_idx.astype(np.int32))


def kernel(**inputs):
    nc = _build_program()
    maps = host_prep(inputs)
    res = run_bass_kernel_spmd(nc, maps, core_ids=list(range(N)))
    accs = [r["acc_out"] for r in res.results]
    return host_combine(accs)


if __name__ == "__main__":
    pass
